# revision 8
# baseline (speedup 1.0000x reference)
"""DeepCrossing (embedding bag lookup + residual MLP) Trainium2 kernel.

Strategy (8 NeuronCores, data-parallel over batch):
  - Batch 4096 is split 512 samples/core. The 1M x 64 embedding table is
    replicated per core (converted to bf16 on host).
  - Per core: indirect-DMA gather of the 512*26*8 looked-up rows (128B
    each), sum-pool over the bag dim (L=8) via PE matmuls against a 0/1
    pooling matrix which directly produce the TRANSPOSED feature matrix
    featT [1664, 512] (features on partitions), so the MLP needs no
    separate transposes.
  - 3 residual blocks + final linear as bf16 matmuls accumulating in f32
    PSUM; bias+ReLU on the scalar engine, residual adds on vector engine.
  - Final sigmoid -> [512] per core -> host concat to [4096, 1] f32.

Self-contained: hardcodes the problem shapes from the task spec.
"""

import numpy as np
import ml_dtypes

# ---- problem constants (from the task spec; do not read files) ----
V = 1_000_000
D = 64
F = 26
L = 8
B = 4096
NCORES = 8
IN_DIM = F * D  # 1664
HIDDENS = (1024, 1024, 512)
P = 128

BF16 = ml_dtypes.bfloat16


# --------------------------------------------------------------------
# Bass kernel builder (parameterized so a tiny config can be simulated)
# --------------------------------------------------------------------
def build_nc(V=V, D=D, F=F, L=L, SPC=B // NCORES, HIDDENS=HIDDENS, debug=False):
    from contextlib import ExitStack

    from concourse import bacc, bass, mybir, tile

    assert D == 64 and L == 8
    IN = F * D
    KT = IN // P                # featT k-tiles (13)
    JC = SPC // 16              # chunks per feature (32)
    CPK = 2 * JC                # chunks per k-tile (64): two features/k-tile
    NCHUNK = F * JC             # total gather chunks (832)
    MTs = [h // P for h in HIDDENS]
    bf16 = mybir.dt.bfloat16
    f32 = mybir.dt.float32
    AF = mybir.ActivationFunctionType
    ALU = mybir.AluOpType

    nc = bacc.Bacc("TRN2", target_bir_lowering=False, debug=False)

    # ---- DRAM I/O (host supplies pre-laid-out arrays) ----
    emb = nc.dram_tensor("emb", [V, D], bf16, kind="ExternalInput")
    idx = nc.dram_tensor("idx", [P, NCHUNK], mybir.dt.int32, kind="ExternalInput")
    S_in = nc.dram_tensor("S", [P, 16], bf16, kind="ExternalInput")
    w1d = [nc.dram_tensor(f"w1_{i}", [P, KT * h], bf16, kind="ExternalInput")
           for i, h in enumerate(HIDDENS)]
    w2d = [nc.dram_tensor(f"w2_{i}", [P, (h // P) * IN], bf16, kind="ExternalInput")
           for i, h in enumerate(HIDDENS)]
    b1d = nc.dram_tensor("b1", [P, sum(MTs)], f32, kind="ExternalInput")
    b2d = nc.dram_tensor("b2", [P, len(HIDDENS) * KT], f32, kind="ExternalInput")
    lwd = nc.dram_tensor("lin_w", [P, KT], bf16, kind="ExternalInput")
    lbd = nc.dram_tensor("lin_b", [1, 1], f32, kind="ExternalInput")
    out_d = nc.dram_tensor("out", [1, SPC], f32, kind="ExternalOutput")
    if debug:
        dbg_g0 = nc.dram_tensor("dbg_g0", [P, D], bf16, kind="ExternalOutput")
        dbg_featT = nc.dram_tensor("dbg_featT", [P, KT * SPC], bf16,
                                   kind="ExternalOutput")
        dbg_feat1 = nc.dram_tensor("dbg_feat1", [P, KT * SPC], bf16,
                                   kind="ExternalOutput")

    with tile.TileContext(nc) as tc, ExitStack() as ctx:
        const = ctx.enter_context(tc.tile_pool(name="const", bufs=1))
        gpool = ctx.enter_context(tc.tile_pool(name="g", bufs=16))
        wpool = ctx.enter_context(tc.tile_pool(name="w", bufs=2))
        apool = ctx.enter_context(tc.tile_pool(name="acts", bufs=1))
        tpool = ctx.enter_context(tc.tile_pool(name="tmp", bufs=2))
        fps_p = ctx.enter_context(tc.tile_pool(name="fps", bufs=3, space="PSUM"))
        mm_p = ctx.enter_context(tc.tile_pool(name="mm", bufs=2, space="PSUM"))
        op_p = ctx.enter_context(tc.tile_pool(name="op", bufs=1, space="PSUM"))

        # ---- constants / small inputs ----
        idx_sb = const.tile([P, NCHUNK], mybir.dt.int32)
        S_sb = const.tile([P, 16], bf16)
        b1_sb = const.tile([P, sum(MTs)], f32)
        b2_sb = const.tile([P, len(HIDDENS) * KT], f32)
        lw_sb = const.tile([P, KT], bf16)
        lb_sb = const.tile([1, 1], f32)
        nc.sync.dma_start(out=idx_sb[:], in_=idx[:])
        nc.sync.dma_start(out=S_sb[:], in_=S_in[:])
        nc.sync.dma_start(out=b1_sb[:], in_=b1d[:])
        nc.sync.dma_start(out=b2_sb[:], in_=b2d[:])
        nc.sync.dma_start(out=lw_sb[:], in_=lwd[:])
        nc.sync.dma_start(out=lb_sb[:], in_=lbd[:])

        # ---- persistent activations ----
        featT = apool.tile([P, KT * SPC], bf16)        # [1664, 512] transposed feats
        hT = apool.tile([P, max(MTs) * SPC], bf16)     # hidden acts, reused per block
        out_sb = apool.tile([1, SPC], f32)

        # ---- weights (streamed per block, double buffered) ----
        w1_sb = []
        w2_sb = []
        for i, h in enumerate(HIDDENS):
            t1 = wpool.tile([P, KT * h], bf16, tag="w1")
            nc.sync.dma_start(out=t1[:], in_=w1d[i][:])
            w1_sb.append(t1)
            t2 = wpool.tile([P, MTs[i] * IN], bf16, tag="w2")
            nc.sync.dma_start(out=t2[:], in_=w2d[i][:])
            w2_sb.append(t2)

        # ---- stage A: gather + pool into featT (transposed) ----
        # HW indirect DMA applies ONE index per destination partition, so each
        # gather op fetches 128 rows -> [128, 64]; one pooling matmul each.
        for kt in range(KT):
            fps = fps_p.tile([P, SPC], f32, tag="fps")
            for c in range(CPK):
                cg = kt * CPK + c
                g = gpool.tile([P, D], bf16, tag="g")
                nc.gpsimd.indirect_dma_start(
                    out=g[:],
                    out_offset=None,
                    in_=emb[:],
                    in_offset=bass.IndirectOffsetOnAxis(
                        ap=idx_sb[:, cg:cg + 1], axis=0),
                )
                f_loc = c // JC          # which of the 2 features in this k-tile
                j = c % JC               # 16-sample chunk
                nc.tensor.matmul(
                    out=fps[f_loc * 64:(f_loc + 1) * 64, j * 16:(j + 1) * 16],
                    lhsT=g[:],
                    rhs=S_sb[:],
                    start=True, stop=True,
                )
                if debug and cg == 0:
                    nc.sync.dma_start(out=dbg_g0[:], in_=g[:])
            nc.vector.tensor_copy(out=featT[:, kt * SPC:(kt + 1) * SPC], in_=fps[:])
        if debug:
            nc.sync.dma_start(out=dbg_featT[:], in_=featT[:])

        # ---- stage B: residual MLP (all on transposed layout) ----
        b1_off = 0
        for i, h in enumerate(HIDDENS):
            MT = MTs[i]
            # hT = relu(w1.T @ featT + b1)
            for m in range(MT):
                ps = mm_p.tile([P, SPC], f32, tag="mm")
                for k in range(KT):
                    nc.tensor.matmul(
                        out=ps[:],
                        lhsT=w1_sb[i][:, k * h + m * P: k * h + (m + 1) * P],
                        rhs=featT[:, k * SPC:(k + 1) * SPC],
                        start=(k == 0), stop=(k == KT - 1),
                    )
                nc.scalar.activation(
                    out=hT[:, m * SPC:(m + 1) * SPC], in_=ps[:], func=AF.Relu,
                    bias=b1_sb[:, b1_off + m: b1_off + m + 1])
            b1_off += MT
            # featT = relu(w2.T @ hT + b2 + featT)
            for k in range(KT):
                ps = mm_p.tile([P, SPC], f32, tag="mm")
                for m in range(MT):
                    nc.tensor.matmul(
                        out=ps[:],
                        lhsT=w2_sb[i][:, m * IN + k * P: m * IN + (k + 1) * P],
                        rhs=hT[:, m * SPC:(m + 1) * SPC],
                        start=(m == 0), stop=(m == MT - 1),
                    )
                tmp = tpool.tile([P, SPC], f32, tag="tmp")
                nc.vector.scalar_tensor_tensor(
                    out=tmp[:], in0=ps[:],
                    scalar=b2_sb[:, i * KT + k: i * KT + k + 1],
                    in1=featT[:, k * SPC:(k + 1) * SPC],
                    op0=ALU.add, op1=ALU.add)
                nc.scalar.activation(
                    out=featT[:, k * SPC:(k + 1) * SPC], in_=tmp[:], func=AF.Relu)

            if debug and i == 0:
                nc.sync.dma_start(out=dbg_feat1[:], in_=featT[:])

        # ---- final linear + sigmoid ----
        ps = op_p.tile([1, SPC], f32, tag="op")
        for k in range(KT):
            nc.tensor.matmul(
                out=ps[:], lhsT=lw_sb[:, k:k + 1],
                rhs=featT[:, k * SPC:(k + 1) * SPC],
                start=(k == 0), stop=(k == KT - 1))
        nc.scalar.activation(out=out_sb[:], in_=ps[:], func=AF.Sigmoid,
                             bias=lb_sb[0:1, 0:1])
        nc.sync.dma_start(out=out_d[:], in_=out_sb[:])

    nc.compile()
    return nc


# --------------------------------------------------------------------
# Host-side input prep
# --------------------------------------------------------------------
def _prep_idx(x_core, F_=F, L_=L):
    """x_core [SPC, F, L] int -> idx matrix [128, F*SPC/16] int32.

    Global chunk c = f*(SPC/16) + j covers samples 16j..16j+16 of feature f;
    within a chunk, partition p = s_rel*8 + l.
    """
    SPC = x_core.shape[0]
    jc = SPC // 16
    m = x_core.transpose(1, 0, 2).reshape(F_, jc, 16, L_).reshape(F_ * jc, 128)
    return np.ascontiguousarray(m.T).astype(np.int32)


def _prep_inputs(inputs, SPC, hiddens=HIDDENS, F_=F, L_=L):
    IN = F_ * D
    KT = IN // P
    MTs = [h // P for h in hiddens]
    x = np.asarray(inputs["x"]).astype(np.int64)
    emb_bf = np.ascontiguousarray(np.asarray(inputs["emb_table"], dtype=np.float32).astype(BF16))
    S = (np.arange(P)[:, None] // L_ == np.arange(16)[None, :]).astype(BF16)

    shared = {"emb": emb_bf, "S": S}
    for i, h in enumerate(hiddens):
        w1 = np.asarray(inputs[f"w1_{i}"], dtype=np.float32)   # [IN, h]
        w2 = np.asarray(inputs[f"w2_{i}"], dtype=np.float32)   # [h, IN]
        shared[f"w1_{i}"] = np.ascontiguousarray(
            w1.reshape(KT, P, h).transpose(1, 0, 2).reshape(P, KT * h).astype(BF16))
        shared[f"w2_{i}"] = np.ascontiguousarray(
            w2.reshape(h // P, P, IN).transpose(1, 0, 2).reshape(P, (h // P) * IN).astype(BF16))
    b1 = np.concatenate([np.asarray(inputs[f"b1_{i}"], dtype=np.float32)
                         .reshape(MTs[i], P).T for i in range(len(hiddens))], axis=1)
    b2 = np.concatenate([np.asarray(inputs[f"b2_{i}"], dtype=np.float32)
                         .reshape(KT, P).T for i in range(len(hiddens))], axis=1)
    shared["b1"] = np.ascontiguousarray(b1)
    shared["b2"] = np.ascontiguousarray(b2)
    shared["lin_w"] = np.ascontiguousarray(
        np.asarray(inputs["lin_w"], dtype=np.float32).reshape(KT, P).T.astype(BF16))
    shared["lin_b"] = np.asarray(inputs["lin_b"], dtype=np.float32).reshape(1, 1)

    ncores = x.shape[0] // SPC
    in_maps = []
    for c in range(ncores):
        m = dict(shared)
        m["idx"] = _prep_idx(x[c * SPC:(c + 1) * SPC], F_, L_)
        in_maps.append(m)
    return in_maps


# --------------------------------------------------------------------
# Public entry point
# --------------------------------------------------------------------
_NC_CACHE = {}


def kernel(**inputs):
    from concourse.bass_utils import run_bass_kernel_spmd

    SPC = B // NCORES
    if "nc" not in _NC_CACHE:
        _NC_CACHE["nc"] = build_nc()
    nc = _NC_CACHE["nc"]
    in_maps = _prep_inputs(inputs, SPC)
    res = run_bass_kernel_spmd(nc, in_maps, core_ids=list(range(NCORES)))
    outs = [np.asarray(r["out"], dtype=np.float32).reshape(SPC) for r in res.results]
    return np.concatenate(outs).reshape(B, 1).astype(np.float32)
